# revision 5
# baseline (speedup 1.0000x reference)
"""Trainium2 Bass kernel: MultiHeadSelfAttention (LayerNorm -> QKV -> masked
softmax attention -> output projection).

Problem shapes: B=4, S=2048, D=512, H=8, DK=64, fp32 I/O.

Sharding: 8 cores = 4 batches x 2 query-halves. Each core computes the full
K/V for its batch and attention outputs for its 1024 queries; no cross-core
communication. SPMD trick: the token order of each core's input is permuted on
the host so that the core's queries are always tokens 0..1023 (one static
program for all cores; attention is permutation-equivariant over keys as long
as the key-padding mask is permuted consistently).

Host prep: LayerNorm (memory-bound elementwise) + the [tok,d]->[d,tok]
transpose run in numpy, so the device receives xnT (bf16, d-major, shipped as
contiguous 128-partition chunks) and does pure matmul/attention work.

Device schedule: the exp of all 16.8M score entries on ScalarE (~1ns/elem) is
the hard floor (~147us), so everything else hides under it:
  - two persistent PSUM score tiles (one per head-half) let scores(c+1)
    overlap exp(c) with no ACT stalls;
  - PV runs 2 chunks behind exp so the FIFO PE queue never blocks on the
    previous pair's PSUM evacuation;
  - V/QK projections for later pairs, the previous pair's PV tail +
    normalization + output-transposes, and 3/4 of the output projection all
    drain through a deferred-work queue paced one item per chunk.

PSUM budget (8 banks): sc0 (2) + sc1 (2) + PV accumulators (3) + proj (1).
"""

import math

import numpy as np
from ml_dtypes import bfloat16 as np_bf16

import concourse.bass as bass
import concourse.tile as tile
from concourse import bacc, mybir
from concourse.bass_utils import run_bass_kernel_spmd
from concourse.masks import make_identity

B, S, D, H, DK = 4, 2048, 512, 8, 64
P = 128                 # partitions
NQ = 1024               # queries per core
NT = S // P             # 16 token tiles / key chunks
DC = D // P             # 4 d-chunks
NQT = NQ // P           # 8 query tiles
PAIRS = H // 2          # 4 head pairs
F32 = mybir.dt.float32
BF16 = mybir.dt.bfloat16
NEG = -1.0e30


def _emit(tc: tile.TileContext, ctx):
    nc = tc.nc

    xnT_d = nc.dram_tensor("xnT", [DC, P, S], BF16, kind="ExternalInput")
    wq_d = nc.dram_tensor("wq", [DC, P, D], BF16, kind="ExternalInput")
    wk_d = nc.dram_tensor("wk", [DC, P, D], BF16, kind="ExternalInput")
    wv_d = nc.dram_tensor("wv", [DC, P, D], BF16, kind="ExternalInput")
    wo_d = nc.dram_tensor("wo", [DC, P, D], BF16, kind="ExternalInput")
    bq_d = nc.dram_tensor("bq", [P, DC], F32, kind="ExternalInput")
    bk_d = nc.dram_tensor("bk", [P, DC], F32, kind="ExternalInput")
    bo_d = nc.dram_tensor("bo", [D], F32, kind="ExternalInput")
    mb_d = nc.dram_tensor("maskb", [P, NT], F32, kind="ExternalInput")
    y_d = nc.dram_tensor("y", [NQ, D], F32, kind="ExternalOutput")

    consts = ctx.enter_context(tc.tile_pool(name="consts", bufs=1))
    big = ctx.enter_context(tc.tile_pool(name="big", bufs=1))
    ptp = ctx.enter_context(tc.tile_pool(name="ptp", bufs=4))
    rlp = ctx.enter_context(tc.tile_pool(name="rlp", bufs=6))
    yout = ctx.enter_context(tc.tile_pool(name="yout", bufs=3))

    ident = consts.tile([P, P], BF16, tag="ident")
    make_identity(nc, ident)
    bq_sb = consts.tile([P, DC], F32, tag="bq")
    nc.sync.dma_start(bq_sb, bq_d[:, :])
    bk_sb = consts.tile([P, DC], F32, tag="bk")
    nc.sync.dma_start(bk_sb, bk_d[:, :])
    mb_sb = consts.tile([P, NT], F32, tag="mb")
    nc.sync.dma_start(mb_sb, mb_d[:, :])
    bo_sb = consts.tile([P, D], F32, tag="bo")
    bo_ap = bo_d[:]
    nc.sync.dma_start(
        bo_sb, bass.AP(tensor=bo_ap.tensor, offset=bo_ap.offset, ap=[[0, P], [1, D]])
    )

    # persistent bf16 operands; DMA order = first-use order (wv, wq, wk, then
    # the first token-half of xnT, the rest, wo last)
    w_sb = {
        name: big.tile([P, DC, D], BF16, tag=f"{name}_sb", name=f"{name}_sb")
        for name in ("wq", "wk", "wv", "wo")
    }
    xnT = big.tile([P, DC, S], BF16, tag="xnT")
    for name, d in (("wv", wv_d), ("wq", wq_d), ("wk", wk_d)):
        for c in range(DC):
            nc.sync.dma_start(w_sb[name][:, c, :], d[c, :, :])
    for half in range(2):
        for c in range(DC):
            nc.sync.dma_start(
                xnT[:, c, half * NQ : (half + 1) * NQ],
                xnT_d[c, :, half * NQ : (half + 1) * NQ],
            )
    for c in range(DC):
        nc.sync.dma_start(w_sb["wo"][:, c, :], wo_d[c, :, :])

    qT = big.tile([P, DC, NQ], BF16, tag="qT")
    kT = big.tile([P, DC, S], BF16, tag="kT")
    vaug = big.tile([P, NT, 8 * 65], BF16, tag="vaug")
    attno = big.tile([P, NQT, D], BF16, tag="attno")
    outT = big.tile([P, DC, NQ], BF16, tag="outT")
    yA = big.tile([P, NQT, D], F32, tag="yA")

    def v_group(pool, t):
        def emit():
            vps = pool.tile([P, 512], F32, tag="pp", name=f"vps{t}")
            for dc in range(DC):
                nc.tensor.matmul(
                    vps,
                    xnT[:, dc, t * P : (t + 1) * P],
                    w_sb["wv"][:, dc, :],
                    start=(dc == 0), stop=(dc == DC - 1),
                )
            vslot = vaug[:, t, :].rearrange("p (h c) -> p h c", h=H)
            nc.vector.tensor_copy(
                out=vslot[:, :, 0:DK],
                in_=vps[:].rearrange("p (h c) -> p h c", h=H),
            )
            nc.vector.memset(vslot[:, :, DK : DK + 1], 1.0)
        return emit

    def qk_groups(pool, p):
        """Emit-closures for pair p's Q and K projections (d-chunk p)."""
        groups = []

        def proj_group(w_name, out_t, bias, gg):
            def emit():
                ps = pool.tile([P, 512], F32, tag="pp", name=f"{w_name}ps{p}_{gg}")
                for dc in range(DC):
                    nc.tensor.matmul(
                        ps,
                        w_sb[w_name][:, dc, p * P : (p + 1) * P],
                        xnT[:, dc, gg * 512 : (gg + 1) * 512],
                        start=(dc == 0), stop=(dc == DC - 1),
                    )
                nc.vector.tensor_scalar_add(
                    out=out_t[:, p, gg * 512 : (gg + 1) * 512], in0=ps,
                    scalar1=bias[:, p : p + 1],
                )
            return emit

        for qg in range(NQ // 512):
            groups.append(proj_group("wq", qT, bq_sb, qg))
        for kg in range(S // 512):
            groups.append(proj_group("wk", kT, bk_sb, kg))
        return groups

    def evac_items(pool, p, pvb):
        """Normalize pair p's PV into attno, then transpose into outT.
        Returned as small closures so they pace into pair p+1's chunk loop."""
        items = []
        rls = []

        def recips():
            for j, bank in enumerate(pvb):
                nslot = 3 if j < 2 else 2
                rl = rlp.tile([P, 3, 2], F32, tag="rl", name=f"rl{p}_{j}")
                lcols = bass.AP(
                    tensor=bank.tensor, offset=bank.offset + 64,
                    ap=[list(x) for x in bank.ap[:1]] + [[130, nslot], [65, 2]],
                )
                nc.vector.reciprocal(out=rl[:, :nslot, :], in_=lcols)
                rls.append(rl)
        items.append(recips)

        def ts_block(q0):
            def emit():
                for qt in range(q0, q0 + 2):
                    bank = pvb[qt // 3]
                    off = (qt % 3) * 130
                    for hs in range(2):
                        nc.vector.tensor_scalar_mul(
                            out=attno[
                                :, qt, (2 * p + hs) * DK : (2 * p + hs + 1) * DK
                            ],
                            in0=bank[:, off + hs * 65 : off + hs * 65 + DK],
                            scalar1=rls[qt // 3][:, qt % 3, hs : hs + 1],
                        )
            return emit
        for q0 in range(0, NQT, 2):
            items.append(ts_block(q0))

        def tr_block(q0):
            def emit():
                for qt in range(q0, q0 + 4):
                    tre = pool.tile([P, P], BF16, tag="pp", name=f"tre{p}_{qt}")
                    nc.tensor.transpose(
                        tre, attno[:, qt, p * P : (p + 1) * P], ident
                    )
                    nc.vector.tensor_copy(
                        out=outT[:, p, qt * P : (qt + 1) * P], in_=tre
                    )
            return emit
        for q0 in range(0, NQT, 4):
            items.append(tr_block(q0))
        return items

    def oprojA_group(pool, qt):
        """Output projection over d-chunks 0..2 (ready before pair 3 ends);
        bias folded in so the tail only adds the last chunk's contribution."""
        def emit():
            ps = pool.tile([P, D], F32, tag="pp", name=f"oA{qt}")
            for dc in range(DC - 1):
                nc.tensor.matmul(
                    ps,
                    outT[:, dc, qt * P : (qt + 1) * P],
                    w_sb["wo"][:, dc, :],
                    start=(dc == 0), stop=(dc == DC - 2),
                )
            nc.vector.tensor_tensor(
                out=yA[:, qt, :], in0=ps, in1=bo_sb, op=mybir.AluOpType.add
            )
        return emit

    # ---------------- attention, everything else in its shadow ----------------
    with tc.tile_pool(name="att", bufs=1, space="PSUM") as att:
        # prologue: V for the first chunks + pair-0 Q/K
        for t in range(2):
            v_group(att, t)()
        for g in qk_groups(att, 0):
            g()

        sc = [
            att.tile([P, NQ], F32, tag=f"sc{hs}", name=f"sc{hs}") for hs in (0, 1)
        ]
        pending = []
        for p in range(PAIRS):
            pvb = [
                att.tile([P, 512], F32, tag="pvb", bufs=3, name=f"pvb{p}_{j}")
                for j in range(3)
            ]
            if p == 0:
                pending += [v_group(att, t) for t in range(2, NT)]
            if p + 1 < PAIRS:
                pending += qk_groups(att, p + 1)
            if p == PAIRS - 1:
                pending += [oprojA_group(att, qt) for qt in range(NQT)]
            pts = [None] * NT
            for c in range(NT):
                # 4 score matmuls; hs-adjacent issue order so the two
                # 64-row tiles (rows 0-63 / 64-127) overlap in the array.
                for qg in range(NQ // 512):
                    for hs in range(2):
                        nc.tensor.matmul(
                            sc[hs][:, qg * 512 : (qg + 1) * 512],
                            kT[hs * DK : (hs + 1) * DK, p, c * P : (c + 1) * P],
                            qT[hs * DK : (hs + 1) * DK, p, qg * 512 : (qg + 1) * 512],
                            start=True, stop=True,
                        )
                pt = ptp.tile([P, 2 * NQ], BF16, tag="pt")
                pts[c] = pt
                for hs in range(2):
                    nc.scalar.activation(
                        out=pt[:, hs * NQ : (hs + 1) * NQ], in_=sc[hs],
                        func=mybir.ActivationFunctionType.Exp,
                        bias=mb_sb[:, c : c + 1], scale=1.0 / math.sqrt(DK),
                    )
                # PV lags exp by 2 chunks: the PE FIFO never stalls on the
                # previous pair's bank evacuation.
                if c >= 2:
                    _pv_chunk(nc, pts[c - 2], vaug, pvb, p, c - 2)
                    pts[c - 2] = None
                # drain deferred work, faster when backlogged
                npop = 2 if len(pending) > NT - c else 1
                for _ in range(min(npop, len(pending))):
                    pending.pop(0)()
            _pv_chunk(nc, pts[NT - 2], vaug, pvb, p, NT - 2)
            if p + 1 < PAIRS:
                # tail chunk + evacuation pace into the next pair's loop
                pending = (
                    [lambda pt=pts[NT - 1], pp=p, pb=pvb: _pv_chunk(
                        nc, pt, vaug, pb, pp, NT - 1)]
                    + evac_items(att, p, pvb)
                    + pending
                )
            else:
                _pv_chunk(nc, pts[NT - 1], vaug, pvb, p, NT - 1)
                while pending:
                    pending.pop(0)()
                for it in evac_items(att, p, pvb):
                    it()

    # ---------------- output projection tail (last d-chunk only) -------------
    with tc.tile_pool(name="projE", bufs=4, space="PSUM") as projE:
        for qt in range(NQT):
            po = projE.tile([P, D], F32, tag="ops")
            nc.tensor.matmul(
                po,
                outT[:, DC - 1, qt * P : (qt + 1) * P],
                w_sb["wo"][:, DC - 1, :],
                start=True, stop=True,
            )
            yt = yout.tile([P, D], F32, tag="yt")
            nc.vector.tensor_tensor(
                out=yt, in0=po, in1=yA[:, qt, :], op=mybir.AluOpType.add
            )
            nc.sync.dma_start(y_d[qt * P : (qt + 1) * P, :], yt)


def _pv_chunk(nc, pt, vaug, pvb, p, c):
    """P@[V|1] matmuls for chunk c of head-pair p: 8 query tiles x 2 heads,
    accumulated over chunks into the packed PSUM banks."""
    for qt in range(NQT):
        bank = pvb[qt // 3]
        off = (qt % 3) * 130
        for hs in range(2):
            h = 2 * p + hs
            # start=True clears has_written for the WHOLE bank, so only the
            # first packed region per bank may use it; the others rely on
            # overwrite-when-bit-clear for their first chunk.
            nc.tensor.matmul(
                bank[:, off + hs * 65 : off + (hs + 1) * 65],
                pt[:, hs * NQ + qt * P : hs * NQ + (qt + 1) * P],
                vaug[:, c, h * 65 : (h + 1) * 65],
                start=(c == 0 and qt % 3 == 0 and hs == 0),
                stop=(c == NT - 1),
                skip_group_check=True,
            )


_NC = None


def _get_nc():
    global _NC
    if _NC is None:
        from contextlib import ExitStack

        nc = bacc.Bacc(None, target_bir_lowering=False)
        with tile.TileContext(nc) as tc, ExitStack() as ctx:
            _emit(tc, ctx)
        nc.compile()
        _NC = nc
    return _NC


def kernel(
    inputs, input_lengths, pos_embed, ln_gamma, ln_beta,
    Wq, bq, Wk, bk, Wv, bv, Wo, bo,
):
    x = np.ascontiguousarray(np.asarray(inputs, np.float32))
    lengths = np.asarray(input_lengths, np.int32)
    g = np.asarray(ln_gamma, np.float32)
    be = np.asarray(ln_beta, np.float32)
    Wq = np.asarray(Wq, np.float32); bq = np.asarray(bq, np.float32)
    Wk = np.asarray(Wk, np.float32); bk = np.asarray(bk, np.float32)
    Wv = np.asarray(Wv, np.float32); bv = np.asarray(bv, np.float32)
    Wo = np.asarray(Wo, np.float32); bo = np.asarray(bo, np.float32)

    # LayerNorm on host (eps=1e-5), fp32, then bf16 d-major per core.
    mu = x.mean(-1, keepdims=True)
    xc = x - mu
    var = np.mean(xc * xc, axis=-1, keepdims=True)
    xn = (xc / np.sqrt(var + 1e-5)) * g + be

    def chunks(w):  # [D, D] -> [DC, P, D] contiguous row chunks of W.T
        return np.ascontiguousarray(w.T.astype(np_bf16).reshape(DC, P, D))

    wq_h, wk_h, wv_h, wo_h = chunks(Wq), chunks(Wk), chunks(Wv), chunks(Wo)
    bq_h = np.ascontiguousarray(bq.reshape(DC, P).T)
    bk_h = np.ascontiguousarray(bk.reshape(DC, P).T)
    # V bias passes through softmax (rows sum to 1) -> fold into output bias.
    bo_h = np.ascontiguousarray(bo + bv @ Wo.T)

    maskb = np.where(np.arange(S)[None, :] < lengths[:, None], 0.0, NEG).astype(
        np.float32
    )

    nc = _get_nc()
    in_maps = []
    core_assign = []
    for b in range(B):
        for gq in range(2):
            order = np.r_[gq * NQ : (gq + 1) * NQ, (1 - gq) * NQ : (2 - gq) * NQ]
            in_maps.append(
                {
                    "xnT": np.ascontiguousarray(
                        xn[b][order].T.astype(np_bf16).reshape(DC, P, S)
                    ),
                    "wq": wq_h, "wk": wk_h, "wv": wv_h, "wo": wo_h,
                    "bq": bq_h, "bk": bk_h, "bo": bo_h,
                    "maskb": np.ascontiguousarray(maskb[b][order].reshape(NT, P).T),
                }
            )
            core_assign.append((b, gq))

    global _LAST_IN_MAPS
    _LAST_IN_MAPS = in_maps
    res = run_bass_kernel_spmd(nc, in_maps, core_ids=list(range(8)))

    y = np.empty((B, S, D), np.float32)
    for i, (b, gq) in enumerate(core_assign):
        y[b, gq * NQ : (gq + 1) * NQ] = res.results[i]["y"]
    return y


# revision 7
# speedup vs baseline: 1.1829x; 1.1829x over previous
"""Trainium2 Bass kernel: MultiHeadSelfAttention (LayerNorm -> QKV -> masked
softmax attention -> output projection).

Problem shapes: B=4, S=2048, D=512, H=8, DK=64, fp32 I/O.

Sharding: 8 cores = 4 batches x 2 query-halves. Each core computes the full
K/V for its batch and attention outputs for its 1024 queries; no cross-core
communication. SPMD trick: the token order of each core's input is permuted on
the host so that the core's queries are always tokens 0..1023 (one static
program for all cores; attention is permutation-equivariant over keys as long
as the key-padding mask is permuted consistently).

Host prep: LayerNorm (memory-bound elementwise) + the [tok,d]->[d,tok]
transpose run in numpy, so the device receives xnT (bf16, d-major, shipped as
contiguous 128-partition chunks) and does pure matmul/attention work.

Device schedule: the exp of all 16.8M score entries on ScalarE (~1ns/elem) is
the hard floor (~147us), so everything else hides under it:
  - two persistent PSUM score tiles (one per head-half) let scores(c+1)
    overlap exp(c) with no ACT stalls;
  - PV runs 2 chunks behind exp so the FIFO PE queue never blocks on the
    previous pair's PSUM evacuation;
  - V/QK projections for later pairs, the previous pair's PV tail +
    normalization + output-transposes, and 3/4 of the output projection all
    drain through a deferred-work queue paced one item per chunk.

PSUM budget (8 banks): sc0 (2) + sc1 (2) + PV accumulators (3) + proj (1).
"""

import math

import numpy as np
from ml_dtypes import bfloat16 as np_bf16

import concourse.bass as bass
import concourse.tile as tile
from concourse import bacc, mybir
from concourse.bass_utils import run_bass_kernel_spmd
from concourse.masks import make_identity

B, S, D, H, DK = 4, 2048, 512, 8, 64
P = 128                 # partitions
NQ = 1024               # queries per core
NT = S // P             # 16 token tiles / key chunks
DC = D // P             # 4 d-chunks
NQT = NQ // P           # 8 query tiles
PAIRS = H // 2          # 4 head pairs
F32 = mybir.dt.float32
BF16 = mybir.dt.bfloat16
NEG = -1.0e30


def _emit(tc: tile.TileContext, ctx):
    nc = tc.nc

    xnT_d = nc.dram_tensor("xnT", [DC, P, S], BF16, kind="ExternalInput")
    wq_d = nc.dram_tensor("wq", [DC, P, D], BF16, kind="ExternalInput")
    wk_d = nc.dram_tensor("wk", [DC, P, D], BF16, kind="ExternalInput")
    wv_d = nc.dram_tensor("wv", [DC, P, D], BF16, kind="ExternalInput")
    wo_d = nc.dram_tensor("wo", [DC, P, D], BF16, kind="ExternalInput")
    bq_d = nc.dram_tensor("bq", [P, DC], F32, kind="ExternalInput")
    bk_d = nc.dram_tensor("bk", [P, DC], F32, kind="ExternalInput")
    bo_d = nc.dram_tensor("bo", [D], F32, kind="ExternalInput")
    mb_d = nc.dram_tensor("maskb", [P, NT], F32, kind="ExternalInput")
    y_d = nc.dram_tensor("y", [NQ, D], F32, kind="ExternalOutput")

    consts = ctx.enter_context(tc.tile_pool(name="consts", bufs=1))
    big = ctx.enter_context(tc.tile_pool(name="big", bufs=1))
    ptp = ctx.enter_context(tc.tile_pool(name="ptp", bufs=4))
    rlp = ctx.enter_context(tc.tile_pool(name="rlp", bufs=6))
    yout = ctx.enter_context(tc.tile_pool(name="yout", bufs=3))

    ident = consts.tile([P, P], BF16, tag="ident")
    make_identity(nc, ident)
    bq_sb = consts.tile([P, DC], F32, tag="bq")
    nc.sync.dma_start(bq_sb, bq_d[:, :])
    bk_sb = consts.tile([P, DC], F32, tag="bk")
    nc.sync.dma_start(bk_sb, bk_d[:, :])
    mb_sb = consts.tile([P, NT], F32, tag="mb")
    nc.sync.dma_start(mb_sb, mb_d[:, :])
    bo_sb = consts.tile([P, D], F32, tag="bo")
    bo_ap = bo_d[:]
    nc.sync.dma_start(
        bo_sb, bass.AP(tensor=bo_ap.tensor, offset=bo_ap.offset, ap=[[0, P], [1, D]])
    )

    # persistent bf16 operands; DMA order = first-use order (wv, wq, wk, then
    # the first token-half of xnT, the rest, wo last)
    w_sb = {
        name: big.tile([P, DC, D], BF16, tag=f"{name}_sb", name=f"{name}_sb")
        for name in ("wq", "wk", "wv", "wo")
    }
    xnT = big.tile([P, DC, S], BF16, tag="xnT")
    for name, d in (("wv", wv_d), ("wq", wq_d), ("wk", wk_d)):
        for c in range(DC):
            nc.sync.dma_start(w_sb[name][:, c, :], d[c, :, :])
    for half in range(2):
        for c in range(DC):
            nc.sync.dma_start(
                xnT[:, c, half * NQ : (half + 1) * NQ],
                xnT_d[c, :, half * NQ : (half + 1) * NQ],
            )
    for c in range(DC):
        nc.sync.dma_start(w_sb["wo"][:, c, :], wo_d[c, :, :])

    qT = big.tile([P, DC, NQ], BF16, tag="qT")
    kT = big.tile([P, DC, S], BF16, tag="kT")
    vaug = big.tile([P, NT, 8 * 65], BF16, tag="vaug")
    attno = big.tile([P, NQT, D], BF16, tag="attno")
    outT = big.tile([P, DC, NQ], BF16, tag="outT")
    yA = big.tile([P, NQT, D], F32, tag="yA")

    def v_group(pool, t):
        def emit():
            vps = pool.tile([P, 512], F32, tag="pp", name=f"vps{t}")
            for dc in range(DC):
                nc.tensor.matmul(
                    vps,
                    xnT[:, dc, t * P : (t + 1) * P],
                    w_sb["wv"][:, dc, :],
                    start=(dc == 0), stop=(dc == DC - 1),
                )
            vslot = vaug[:, t, :].rearrange("p (h c) -> p h c", h=H)
            nc.vector.tensor_copy(
                out=vslot[:, :, 0:DK],
                in_=vps[:].rearrange("p (h c) -> p h c", h=H),
            )
            nc.vector.memset(vslot[:, :, DK : DK + 1], 1.0)
        return emit

    def qk_groups(pool, p):
        """Emit-closures for pair p's Q and K projections (d-chunk p)."""
        groups = []

        def proj_group(w_name, out_t, bias, gg):
            def emit():
                ps = pool.tile([P, 512], F32, tag="pp", name=f"{w_name}ps{p}_{gg}")
                for dc in range(DC):
                    nc.tensor.matmul(
                        ps,
                        w_sb[w_name][:, dc, p * P : (p + 1) * P],
                        xnT[:, dc, gg * 512 : (gg + 1) * 512],
                        start=(dc == 0), stop=(dc == DC - 1),
                    )
                nc.vector.tensor_scalar_add(
                    out=out_t[:, p, gg * 512 : (gg + 1) * 512], in0=ps,
                    scalar1=bias[:, p : p + 1],
                )
            return emit

        for qg in range(NQ // 512):
            groups.append(proj_group("wq", qT, bq_sb, qg))
        for kg in range(S // 512):
            groups.append(proj_group("wk", kT, bk_sb, kg))
        return groups

    def evac_items(pool, p, pvb):
        """Normalize pair p's PV into attno, then transpose into outT.
        Returned as small closures so they pace into pair p+1's chunk loop."""
        items = []
        rls = []

        def recips():
            for j, bank in enumerate(pvb):
                nslot = 3 if j < 2 else 2
                rl = rlp.tile([P, 3, 2], F32, tag="rl", name=f"rl{p}_{j}")
                lcols = bass.AP(
                    tensor=bank.tensor, offset=bank.offset + 64,
                    ap=[list(x) for x in bank.ap[:1]] + [[130, nslot], [65, 2]],
                )
                nc.vector.reciprocal(out=rl[:, :nslot, :], in_=lcols)
                rls.append(rl)
        items.append(recips)

        def ts_block(q0):
            def emit():
                for qt in range(q0, q0 + 2):
                    bank = pvb[qt // 3]
                    off = (qt % 3) * 130
                    for hs in range(2):
                        nc.vector.tensor_scalar_mul(
                            out=attno[
                                :, qt, (2 * p + hs) * DK : (2 * p + hs + 1) * DK
                            ],
                            in0=bank[:, off + hs * 65 : off + hs * 65 + DK],
                            scalar1=rls[qt // 3][:, qt % 3, hs : hs + 1],
                        )
            return emit
        for q0 in range(0, NQT, 2):
            items.append(ts_block(q0))

        def tr_block(q0):
            def emit():
                for qt in range(q0, q0 + 4):
                    tre = pool.tile([P, P], BF16, tag="pp", name=f"tre{p}_{qt}")
                    nc.tensor.transpose(
                        tre, attno[:, qt, p * P : (p + 1) * P], ident
                    )
                    nc.vector.tensor_copy(
                        out=outT[:, p, qt * P : (qt + 1) * P], in_=tre
                    )
            return emit
        for q0 in range(0, NQT, 4):
            items.append(tr_block(q0))
        return items

    def oprojA_group(pool, qt):
        """Output projection over d-chunks 0..2 (ready before pair 3 ends);
        bias folded in so the tail only adds the last chunk's contribution."""
        def emit():
            ps = pool.tile([P, D], F32, tag="pp", name=f"oA{qt}")
            for dc in range(DC - 1):
                nc.tensor.matmul(
                    ps,
                    outT[:, dc, qt * P : (qt + 1) * P],
                    w_sb["wo"][:, dc, :],
                    start=(dc == 0), stop=(dc == DC - 2),
                )
            nc.vector.tensor_tensor(
                out=yA[:, qt, :], in0=ps, in1=bo_sb, op=mybir.AluOpType.add
            )
        return emit

    # prologue: V for the first chunks + pair-0 Q/K, own pool for pipelining
    with tc.tile_pool(name="prol", bufs=3, space="PSUM") as prol:
        for t in range(2):
            v_group(prol, t)()
        for g in qk_groups(prol, 0):
            g()

    # ---------------- attention, everything else in its shadow ----------------
    with tc.tile_pool(name="att", bufs=1, space="PSUM") as att:
        sc = [
            att.tile([P, NQ], F32, tag=f"sc{hs}", name=f"sc{hs}") for hs in (0, 1)
        ]
        pending = []
        for p in range(PAIRS):
            pvb = [
                att.tile([P, 512], F32, tag="pvb", bufs=3, name=f"pvb{p}_{j}")
                for j in range(3)
            ]
            if p == 0:
                pending += [v_group(att, t) for t in range(2, NT)]
            if p + 1 < PAIRS:
                pending += qk_groups(att, p + 1)
            if p == PAIRS - 1:
                pending += [oprojA_group(att, qt) for qt in range(NQT)]
            pts = [None] * NT
            for c in range(NT):
                # 4 score matmuls; hs-adjacent issue order so the two
                # 64-row tiles (rows 0-63 / 64-127) overlap in the array.
                for qg in range(NQ // 512):
                    for hs in range(2):
                        nc.tensor.matmul(
                            sc[hs][:, qg * 512 : (qg + 1) * 512],
                            kT[hs * DK : (hs + 1) * DK, p, c * P : (c + 1) * P],
                            qT[hs * DK : (hs + 1) * DK, p, qg * 512 : (qg + 1) * 512],
                            start=True, stop=True,
                        )
                pt = ptp.tile([P, 2 * NQ], BF16, tag="pt")
                pts[c] = pt
                for hs in range(2):
                    nc.scalar.activation(
                        out=pt[:, hs * NQ : (hs + 1) * NQ], in_=sc[hs],
                        func=mybir.ActivationFunctionType.Exp,
                        bias=mb_sb[:, c : c + 1], scale=1.0 / math.sqrt(DK),
                    )
                # PV for the previous chunk keeps PE busy under this exp
                if c > 0:
                    _pv_chunk(nc, pts[c - 1], vaug, pvb, p, c - 1)
                    pts[c - 1] = None
                # drain deferred work, faster when backlogged
                npop = 2 if len(pending) > NT - c else 1
                for _ in range(min(npop, len(pending))):
                    pending.pop(0)()
            _pv_chunk(nc, pts[NT - 1], vaug, pvb, p, NT - 1)
            while pending:
                pending.pop(0)()
            for it in evac_items(att, p, pvb):
                it()

    # ---------------- output projection tail (last d-chunk only) -------------
    with tc.tile_pool(name="projE", bufs=4, space="PSUM") as projE:
        for qt in range(NQT):
            po = projE.tile([P, D], F32, tag="ops")
            nc.tensor.matmul(
                po,
                outT[:, DC - 1, qt * P : (qt + 1) * P],
                w_sb["wo"][:, DC - 1, :],
                start=True, stop=True,
            )
            yt = yout.tile([P, D], F32, tag="yt")
            nc.vector.tensor_tensor(
                out=yt, in0=po, in1=yA[:, qt, :], op=mybir.AluOpType.add
            )
            nc.sync.dma_start(y_d[qt * P : (qt + 1) * P, :], yt)


def _pv_chunk(nc, pt, vaug, pvb, p, c):
    """P@[V|1] matmuls for chunk c of head-pair p: 8 query tiles x 2 heads,
    accumulated over chunks into the packed PSUM banks."""
    for qt in range(NQT):
        bank = pvb[qt // 3]
        off = (qt % 3) * 130
        for hs in range(2):
            h = 2 * p + hs
            # start=True clears has_written for the WHOLE bank, so only the
            # first packed region per bank may use it; the others rely on
            # overwrite-when-bit-clear for their first chunk.
            nc.tensor.matmul(
                bank[:, off + hs * 65 : off + (hs + 1) * 65],
                pt[:, hs * NQ + qt * P : hs * NQ + (qt + 1) * P],
                vaug[:, c, h * 65 : (h + 1) * 65],
                start=(c == 0 and qt % 3 == 0 and hs == 0),
                stop=(c == NT - 1),
                skip_group_check=True,
            )


_NC = None


def _get_nc():
    global _NC
    if _NC is None:
        from contextlib import ExitStack

        nc = bacc.Bacc(None, target_bir_lowering=False)
        with tile.TileContext(nc) as tc, ExitStack() as ctx:
            _emit(tc, ctx)
        nc.compile()
        _NC = nc
    return _NC


def kernel(
    inputs, input_lengths, pos_embed, ln_gamma, ln_beta,
    Wq, bq, Wk, bk, Wv, bv, Wo, bo,
):
    x = np.ascontiguousarray(np.asarray(inputs, np.float32))
    lengths = np.asarray(input_lengths, np.int32)
    g = np.asarray(ln_gamma, np.float32)
    be = np.asarray(ln_beta, np.float32)
    Wq = np.asarray(Wq, np.float32); bq = np.asarray(bq, np.float32)
    Wk = np.asarray(Wk, np.float32); bk = np.asarray(bk, np.float32)
    Wv = np.asarray(Wv, np.float32); bv = np.asarray(bv, np.float32)
    Wo = np.asarray(Wo, np.float32); bo = np.asarray(bo, np.float32)

    # LayerNorm on host (eps=1e-5), fp32, then bf16 d-major per core.
    mu = x.mean(-1, keepdims=True)
    xc = x - mu
    var = np.mean(xc * xc, axis=-1, keepdims=True)
    xn = (xc / np.sqrt(var + 1e-5)) * g + be

    def chunks(w):  # [D, D] -> [DC, P, D] contiguous row chunks of W.T
        return np.ascontiguousarray(w.T.astype(np_bf16).reshape(DC, P, D))

    wq_h, wk_h, wv_h, wo_h = chunks(Wq), chunks(Wk), chunks(Wv), chunks(Wo)
    bq_h = np.ascontiguousarray(bq.reshape(DC, P).T)
    bk_h = np.ascontiguousarray(bk.reshape(DC, P).T)
    # V bias passes through softmax (rows sum to 1) -> fold into output bias.
    bo_h = np.ascontiguousarray(bo + bv @ Wo.T)

    maskb = np.where(np.arange(S)[None, :] < lengths[:, None], 0.0, NEG).astype(
        np.float32
    )

    nc = _get_nc()
    in_maps = []
    core_assign = []
    for b in range(B):
        for gq in range(2):
            order = np.r_[gq * NQ : (gq + 1) * NQ, (1 - gq) * NQ : (2 - gq) * NQ]
            in_maps.append(
                {
                    "xnT": np.ascontiguousarray(
                        xn[b][order].T.astype(np_bf16).reshape(DC, P, S)
                    ),
                    "wq": wq_h, "wk": wk_h, "wv": wv_h, "wo": wo_h,
                    "bq": bq_h, "bk": bk_h, "bo": bo_h,
                    "maskb": np.ascontiguousarray(maskb[b][order].reshape(NT, P).T),
                }
            )
            core_assign.append((b, gq))

    global _LAST_IN_MAPS
    _LAST_IN_MAPS = in_maps
    res = run_bass_kernel_spmd(nc, in_maps, core_ids=list(range(8)))

    y = np.empty((B, S, D), np.float32)
    for i, (b, gq) in enumerate(core_assign):
        y[b, gq * NQ : (gq + 1) * NQ] = res.results[i]["y"]
    return y


# revision 14
# speedup vs baseline: 1.2695x; 1.0732x over previous
"""Trainium2 Bass kernel: MultiHeadSelfAttention (LayerNorm -> QKV -> masked
softmax attention -> output projection).

Problem shapes: B=4, S=2048, D=512, H=8, DK=64, fp32 I/O.

Sharding: 8 cores = 4 batches x 2 query-halves. Each core computes the full
K/V for its batch and attention outputs for its 1024 queries; no cross-core
communication. SPMD trick: the token order of each core's input is permuted on
the host so that the core's queries are always tokens 0..1023 (one static
program for all cores; attention is permutation-equivariant over keys as long
as the key-padding mask is permuted consistently).

Host prep: LayerNorm (memory-bound elementwise) + the [tok,d]->[d,tok]
transpose run in numpy, so the device receives xnT (bf16, d-major) and does
pure matmul/attention work.

Device schedule: the exp of all 16.8M score entries on ScalarE (~1ns/elem) is
the hard floor (~147us); everything else hides under it:
  - warmup matmuls during the input DMAs keep the PE HAM clock-gate at
    2.4 GHz from the first real matmul;
  - weights stream on the ScalarE HWDGE queue (after a dummy exp that
    preloads the ACT table), xnT on the SP queue -> parallel dispatch;
  - two persistent PSUM score tiles let scores(c+1) overlap exp(c);
  - PV runs 2 chunks behind exp; the previous pair's PV tail lands at c=0
    and its whole evacuation (normalize + output-transpose) bursts at c=1
    of the next pair, so the PE FIFO never stalls at a pair boundary;
  - V/QK projections for later pairs and 3/4 of the output projection
    drain through a deferred-work queue paced into the exp shadow;
  - the last pair ends in a per-qt pipeline: normalize -> transpose ->
    final output-projection chunk -> bias add -> DMA out.

PSUM budget (8 banks): sc0 (2) + sc1 (2) + PV accumulators (3) + proj (1).
"""

import math

import numpy as np
from ml_dtypes import bfloat16 as np_bf16

import concourse.bass as bass
import concourse.tile as tile
from concourse import bacc, mybir
from concourse.bass_utils import run_bass_kernel_spmd
from concourse.masks import make_identity

B, S, D, H, DK = 4, 2048, 512, 8, 64
P = 128                 # partitions
NQ = 1024               # queries per core
NT = S // P             # 16 token tiles / key chunks
DC = D // P             # 4 d-chunks
NQT = NQ // P           # 8 query tiles
PAIRS = H // 2          # 4 head pairs
F32 = mybir.dt.float32
BF16 = mybir.dt.bfloat16
NEG = -1.0e30
N_WARM = 20             # warmup matmuls to lift HAM to 2.4 GHz during DMAs


def _emit(tc: tile.TileContext, ctx):
    nc = tc.nc

    xnT_d = nc.dram_tensor("xnT", [DC, P, S], BF16, kind="ExternalInput")
    wq_d = nc.dram_tensor("wq", [DC, P, D], BF16, kind="ExternalInput")
    wk_d = nc.dram_tensor("wk", [DC, P, D], BF16, kind="ExternalInput")
    wv_d = nc.dram_tensor("wv", [DC, P, D], BF16, kind="ExternalInput")
    wo_d = nc.dram_tensor("wo", [DC, P, D], BF16, kind="ExternalInput")
    bq_d = nc.dram_tensor("bq", [P, DC], F32, kind="ExternalInput")
    bk_d = nc.dram_tensor("bk", [P, DC], F32, kind="ExternalInput")
    bo_d = nc.dram_tensor("bo", [D], F32, kind="ExternalInput")
    mb_d = nc.dram_tensor("maskb", [P, NT], F32, kind="ExternalInput")
    y_d = nc.dram_tensor("y", [NQ, D], F32, kind="ExternalOutput")

    consts = ctx.enter_context(tc.tile_pool(name="consts", bufs=1))
    big = ctx.enter_context(tc.tile_pool(name="big", bufs=1))
    ptp = ctx.enter_context(tc.tile_pool(name="ptp", bufs=4))
    rlp = ctx.enter_context(tc.tile_pool(name="rlp", bufs=6))
    yout = ctx.enter_context(tc.tile_pool(name="yout", bufs=3))

    # dummy exp first on the ACT queue: preloads the exp table-set (~2.7us)
    # while the DMAs stream
    zz = consts.tile([P, 512], BF16, tag="zz")
    nc.vector.memset(zz, 0.0)
    dume = consts.tile([P, 1], BF16, tag="dume")
    nc.scalar.activation(
        out=dume, in_=zz[:, 0:1], func=mybir.ActivationFunctionType.Exp
    )

    # weights on the ScalarE HWDGE queue (idle until the first real exp)
    w_sb = {
        name: big.tile([P, DC, D], BF16, tag=f"{name}_sb", name=f"{name}_sb")
        for name in ("wq", "wk", "wv", "wo")
    }
    for name, d in (("wv", wv_d), ("wq", wq_d), ("wk", wk_d), ("wo", wo_d)):
        nc.scalar.dma_start(
            w_sb[name][:, :, :], d[:, :, :].rearrange("c p d -> p c d")
        )

    bq_sb = consts.tile([P, DC], F32, tag="bq")
    nc.sync.dma_start(bq_sb, bq_d[:, :])
    bk_sb = consts.tile([P, DC], F32, tag="bk")
    nc.sync.dma_start(bk_sb, bk_d[:, :])
    mb_sb = consts.tile([P, NT], F32, tag="mb")
    nc.sync.dma_start(mb_sb, mb_d[:, :])
    bo_sb = consts.tile([P, D], F32, tag="bo")
    bo_ap = bo_d[:]
    nc.sync.dma_start(
        bo_sb, bass.AP(tensor=bo_ap.tensor, offset=bo_ap.offset, ap=[[0, P], [1, D]])
    )
    # xnT quarters on the SP queue: V/Q/K consumers unblock incrementally
    xnT = big.tile([P, DC, S], BF16, tag="xnT")
    for tg in range(4):
        nc.sync.dma_start(
            xnT[:, :, tg * 512 : (tg + 1) * 512],
            xnT_d[:, :, tg * 512 : (tg + 1) * 512].rearrange("c p s -> p c s"),
        )

    ident = consts.tile([P, P], BF16, tag="ident")
    make_identity(nc, ident)

    qT = big.tile([P, DC, NQ], BF16, tag="qT")
    kT = big.tile([P, DC, S], BF16, tag="kT")
    vaug = big.tile([P, NT, 8 * 65], BF16, tag="vaug")
    attno = big.tile([P, NQT, D], BF16, tag="attno")
    outT = big.tile([P, DC, NQ], BF16, tag="outT")
    yA = big.tile([P, NQT, D], F32, tag="yA")

    def v_group(pool, t):
        def emit():
            vps = pool.tile([P, 512], F32, tag="pp", name=f"vps{t}")
            for dc in range(DC):
                nc.tensor.matmul(
                    vps,
                    xnT[:, dc, t * P : (t + 1) * P],
                    w_sb["wv"][:, dc, :],
                    start=(dc == 0), stop=(dc == DC - 1),
                )
            vslot = vaug[:, t, :].rearrange("p (h c) -> p h c", h=H)
            nc.vector.tensor_copy(
                out=vslot[:, :, 0:DK],
                in_=vps[:].rearrange("p (h c) -> p h c", h=H),
            )
            nc.vector.memset(vslot[:, :, DK : DK + 1], 1.0)
        return emit

    def qk_groups(pool, p):
        """Emit-closures for pair p's Q and K projections (d-chunk p)."""
        groups = []

        def proj_group(w_name, out_t, bias, gg):
            def emit():
                ps = pool.tile([P, 512], F32, tag="pp", name=f"{w_name}ps{p}_{gg}")
                for dc in range(DC):
                    nc.tensor.matmul(
                        ps,
                        w_sb[w_name][:, dc, p * P : (p + 1) * P],
                        xnT[:, dc, gg * 512 : (gg + 1) * 512],
                        start=(dc == 0), stop=(dc == DC - 1),
                    )
                nc.vector.tensor_scalar_add(
                    out=out_t[:, p, gg * 512 : (gg + 1) * 512], in0=ps,
                    scalar1=bias[:, p : p + 1],
                )
            return emit

        for qg in range(NQ // 512):
            groups.append(proj_group("wq", qT, bq_sb, qg))
        for kg in range(S // 512):
            groups.append(proj_group("wk", kT, bk_sb, kg))
        return groups

    def recips(p, pvb):
        rls = []
        for j, bank in enumerate(pvb):
            nslot = 3 if j < 2 else 2
            rl = rlp.tile([P, 3, 2], F32, tag="rl", name=f"rl{p}_{j}")
            lcols = (
                bank[:, 0 : nslot * 130]
                .rearrange("p (s h t) -> p s h t", s=nslot, h=2)[:, :, :, 64:65]
                .rearrange("p s h t -> p (s h t)")
            )
            nc.vector.reciprocal(
                out=rl[:, :nslot, :].rearrange("p s h -> p (s h)"), in_=lcols
            )
            rls.append(rl)
        return rls

    def evac_qt(pool, p, pvb, rls, qt):
        """Normalize one query tile of pair p into attno + transpose to outT."""
        bank = pvb[qt // 3]
        off = (qt % 3) * 130
        for hs in range(2):
            nc.vector.tensor_scalar_mul(
                out=attno[:, qt, (2 * p + hs) * DK : (2 * p + hs + 1) * DK],
                in0=bank[:, off + hs * 65 : off + hs * 65 + DK],
                scalar1=rls[qt // 3][:, qt % 3, hs : hs + 1],
            )
        tre = pool.tile([P, P], BF16, tag="pp", name=f"tre{p}_{qt}")
        nc.tensor.transpose(tre, attno[:, qt, p * P : (p + 1) * P], ident)
        nc.vector.tensor_copy(out=outT[:, p, qt * P : (qt + 1) * P], in_=tre)

    def oprojA_group(pool, qt):
        """Output projection over d-chunks 0..2 (ready before pair 3 ends);
        bias folded in so the tail only adds the last chunk's contribution."""
        def emit():
            ps = pool.tile([P, D], F32, tag="pp", name=f"oA{qt}")
            for dc in range(DC - 1):
                nc.tensor.matmul(
                    ps,
                    outT[:, dc, qt * P : (qt + 1) * P],
                    w_sb["wo"][:, dc, :],
                    start=(dc == 0), stop=(dc == DC - 2),
                )
            nc.vector.tensor_tensor(
                out=yA[:, qt, :], in0=ps, in1=bo_sb, op=mybir.AluOpType.add
            )
        return emit

    # prologue: warmup + V for the first chunks + pair-0 Q/K
    with tc.tile_pool(name="prol", bufs=3, space="PSUM") as prol:
        wps = prol.tile([P, 512], F32, tag="warm", bufs=1)
        for _ in range(N_WARM):
            nc.tensor.matmul(wps, zz[:, 0:P], zz, start=True, stop=True)
        for t in range(2):
            v_group(prol, t)()
        for g in qk_groups(prol, 0):
            g()

    # ---------------- attention, everything else in its shadow ----------------
    with tc.tile_pool(name="att", bufs=1, space="PSUM") as att:
        sc = [
            att.tile([P, NQ], F32, tag=f"sc{hs}", name=f"sc{hs}") for hs in (0, 1)
        ]
        pending = []
        carry = None
        for p in range(PAIRS):
            pvb = None  # allocated at c==2, AFTER the previous pair's
            # evacuation burst is emitted — the 3-buffer ring's WAR edges
            # only see already-emitted readers of the old tiles.
            if p == 0:
                pending += [v_group(att, t) for t in range(2, NT)]
            if p + 1 < PAIRS:
                pending += qk_groups(att, p + 1)
            pts = [None] * NT
            for c in range(NT):
                # 4 score matmuls; hs-adjacent issue order so the two
                # 64-row tiles (rows 0-63 / 64-127) overlap in the array.
                for qg in range(NQ // 512):
                    for hs in range(2):
                        nc.tensor.matmul(
                            sc[hs][:, qg * 512 : (qg + 1) * 512],
                            kT[hs * DK : (hs + 1) * DK, p, c * P : (c + 1) * P],
                            qT[hs * DK : (hs + 1) * DK, p, qg * 512 : (qg + 1) * 512],
                            start=True, stop=True,
                        )
                pt = ptp.tile([P, 2 * NQ], BF16, tag="pt")
                pts[c] = pt
                for hs in range(2):
                    nc.scalar.activation(
                        out=pt[:, hs * NQ : (hs + 1) * NQ], in_=sc[hs],
                        func=mybir.ActivationFunctionType.Exp,
                        bias=mb_sb[:, c : c + 1], scale=1.0 / math.sqrt(DK),
                    )
                if carry is not None:
                    if c == 0:
                        # previous pair's PV tail chunk
                        _pv_chunk(nc, carry[0], vaug, carry[1], carry[2], NT - 1)
                    elif c == 1:
                        # previous pair's evacuation burst
                        cp, cb = carry[2], carry[1]
                        crl = recips(cp, cb)
                        for qt in range(NQT):
                            evac_qt(att, cp, cb, crl, qt)
                        carry = None
                        if p == PAIRS - 1:
                            # outT d-chunks 0..2 are now fully written
                            pending = [
                                oprojA_group(att, qt) for qt in range(NQT)
                            ] + pending
                # PV lags exp by 2 chunks: the PE FIFO never stalls on the
                # previous pair's bank evacuation.
                if c >= 2:
                    if pvb is None:
                        pvb = [
                            att.tile([P, 512], F32, tag="pvb", bufs=3,
                                     name=f"pvb{p}_{j}")
                            for j in range(3)
                        ]
                    _pv_chunk(nc, pts[c - 2], vaug, pvb, p, c - 2)
                    pts[c - 2] = None
                # drain deferred work, faster when backlogged
                npop = 2 if (p == PAIRS - 1 or len(pending) > NT - 1 - c) else 1
                for _ in range(min(npop, len(pending))):
                    pending.pop(0)()
            _pv_chunk(nc, pts[NT - 2], vaug, pvb, p, NT - 2)
            if p + 1 < PAIRS:
                carry = (pts[NT - 1], pvb, p)
            else:
                # last pair: tail chunk, then per-qt pipeline to the output
                _pv_chunk(nc, pts[NT - 1], vaug, pvb, p, NT - 1)
                rls = recips(p, pvb)
                for qt in range(NQT):
                    evac_qt(att, p, pvb, rls, qt)
                    po = att.tile([P, D], F32, tag="pp", name=f"po{qt}")
                    nc.tensor.matmul(
                        po,
                        outT[:, DC - 1, qt * P : (qt + 1) * P],
                        w_sb["wo"][:, DC - 1, :],
                        start=True, stop=True,
                    )
                    yt = yout.tile([P, D], F32, tag="yt")
                    nc.vector.tensor_tensor(
                        out=yt, in0=po, in1=yA[:, qt, :], op=mybir.AluOpType.add
                    )
                    nc.sync.dma_start(y_d[qt * P : (qt + 1) * P, :], yt)


def _pv_chunk(nc, pt, vaug, pvb, p, c):
    """P@[V|1] matmuls for chunk c of head-pair p: 8 query tiles x 2 heads,
    accumulated over chunks into the packed PSUM banks."""
    for qt in range(NQT):
        bank = pvb[qt // 3]
        off = (qt % 3) * 130
        for hs in range(2):
            h = 2 * p + hs
            # start=True clears has_written for the WHOLE bank, so only the
            # first packed region per bank may use it; the others rely on
            # overwrite-when-bit-clear for their first chunk.
            nc.tensor.matmul(
                bank[:, off + hs * 65 : off + (hs + 1) * 65],
                pt[:, hs * NQ + qt * P : hs * NQ + (qt + 1) * P],
                vaug[:, c, h * 65 : (h + 1) * 65],
                start=(c == 0 and qt % 3 == 0 and hs == 0),
                stop=(c == NT - 1),
                skip_group_check=True,
            )


_NC = None


def _get_nc():
    global _NC
    if _NC is None:
        from contextlib import ExitStack

        nc = bacc.Bacc(None, target_bir_lowering=False)
        with tile.TileContext(nc) as tc, ExitStack() as ctx:
            _emit(tc, ctx)
        nc.compile()
        _NC = nc
    return _NC


def kernel(
    inputs, input_lengths, pos_embed, ln_gamma, ln_beta,
    Wq, bq, Wk, bk, Wv, bv, Wo, bo,
):
    x = np.ascontiguousarray(np.asarray(inputs, np.float32))
    lengths = np.asarray(input_lengths, np.int32)
    g = np.asarray(ln_gamma, np.float32)
    be = np.asarray(ln_beta, np.float32)
    Wq = np.asarray(Wq, np.float32); bq = np.asarray(bq, np.float32)
    Wk = np.asarray(Wk, np.float32); bk = np.asarray(bk, np.float32)
    Wv = np.asarray(Wv, np.float32); bv = np.asarray(bv, np.float32)
    Wo = np.asarray(Wo, np.float32); bo = np.asarray(bo, np.float32)

    # LayerNorm on host (eps=1e-5), fp32, then bf16 d-major per core.
    mu = x.mean(-1, keepdims=True)
    xc = x - mu
    var = np.mean(xc * xc, axis=-1, keepdims=True)
    xn = (xc / np.sqrt(var + 1e-5)) * g + be

    def chunks(w):  # [D, D] -> [DC, P, D] contiguous row chunks of W.T
        return np.ascontiguousarray(w.T.astype(np_bf16).reshape(DC, P, D))

    wq_h, wk_h, wv_h, wo_h = chunks(Wq), chunks(Wk), chunks(Wv), chunks(Wo)
    bq_h = np.ascontiguousarray(bq.reshape(DC, P).T)
    bk_h = np.ascontiguousarray(bk.reshape(DC, P).T)
    # V bias passes through softmax (rows sum to 1) -> fold into output bias.
    bo_h = np.ascontiguousarray(bo + bv @ Wo.T)

    maskb = np.where(np.arange(S)[None, :] < lengths[:, None], 0.0, NEG).astype(
        np.float32
    )

    nc = _get_nc()
    in_maps = []
    core_assign = []
    for b in range(B):
        for gq in range(2):
            order = np.r_[gq * NQ : (gq + 1) * NQ, (1 - gq) * NQ : (2 - gq) * NQ]
            in_maps.append(
                {
                    "xnT": np.ascontiguousarray(
                        xn[b][order].T.astype(np_bf16).reshape(DC, P, S)
                    ),
                    "wq": wq_h, "wk": wk_h, "wv": wv_h, "wo": wo_h,
                    "bq": bq_h, "bk": bk_h, "bo": bo_h,
                    "maskb": np.ascontiguousarray(maskb[b][order].reshape(NT, P).T),
                }
            )
            core_assign.append((b, gq))

    global _LAST_IN_MAPS
    _LAST_IN_MAPS = in_maps
    res = run_bass_kernel_spmd(nc, in_maps, core_ids=list(range(8)))

    y = np.empty((B, S, D), np.float32)
    for i, (b, gq) in enumerate(core_assign):
        y[b, gq * NQ : (gq + 1) * NQ] = res.results[i]["y"]
    return y


# revision 19
# speedup vs baseline: 1.3038x; 1.0270x over previous
"""Trainium2 Bass kernel: MultiHeadSelfAttention (LayerNorm -> QKV -> masked
softmax attention -> output projection).

Problem shapes: B=4, S=2048, D=512, H=8, DK=64, fp32 I/O.

Sharding: 8 cores = 4 batches x 2 query-halves. Each core computes the full
K/V for its batch and attention outputs for its 1024 queries; no cross-core
communication. SPMD trick: the token order of each core's input is permuted on
the host so that the core's queries are always tokens 0..1023 (one static
program for all cores; attention is permutation-equivariant over keys as long
as the key-padding mask is permuted consistently).

Host prep: LayerNorm (memory-bound elementwise) + the [tok,d]->[d,tok]
transpose run in numpy, so the device receives xnT (bf16, d-major) and does
pure matmul/attention work.

Device schedule: the exp of all 16.8M score entries on ScalarE (~1ns/elem) is
the hard floor (~147us); everything else hides under it:
  - warmup matmuls during the input DMAs keep the PE HAM clock-gate at
    2.4 GHz from the first real matmul;
  - weights stream on the ScalarE HWDGE queue (after a dummy exp that
    preloads the ACT table), xnT on the SP queue -> parallel dispatch;
  - two persistent PSUM score tiles let scores(c+1) overlap exp(c);
  - PV runs 2 chunks behind exp; the previous pair's PV tail lands at c=0
    and its whole evacuation (normalize + output-transpose) bursts at c=1
    of the next pair, so the PE FIFO never stalls at a pair boundary;
  - V/QK projections for later pairs and 3/4 of the output projection
    drain through a deferred-work queue paced into the exp shadow;
  - the last pair ends in a per-qt pipeline: normalize -> transpose ->
    final output-projection chunk -> bias add -> DMA out.

PSUM budget (8 banks): sc0 (2) + sc1 (2) + PV accumulators (3) + proj (1).
"""

import math

import numpy as np
from ml_dtypes import bfloat16 as np_bf16

import concourse.bass as bass
import concourse.tile as tile
from concourse import bacc, mybir
from concourse.bass_utils import run_bass_kernel_spmd
from concourse.masks import make_identity

B, S, D, H, DK = 4, 2048, 512, 8, 64
P = 128                 # partitions
NQ = 1024               # queries per core
NT = S // P             # 16 token tiles / key chunks
DC = D // P             # 4 d-chunks
NQT = NQ // P           # 8 query tiles
PAIRS = H // 2          # 4 head pairs
F32 = mybir.dt.float32
BF16 = mybir.dt.bfloat16
NEG = -1.0e30
N_WARM = 30             # warmup matmuls to lift HAM to 2.4 GHz during DMAs


def _emit(tc: tile.TileContext, ctx):
    nc = tc.nc

    xnT_d = nc.dram_tensor("xnT", [DC, P, S], BF16, kind="ExternalInput")
    wq_d = nc.dram_tensor("wq", [DC, P, D], BF16, kind="ExternalInput")
    wk_d = nc.dram_tensor("wk", [DC, P, D], BF16, kind="ExternalInput")
    wv_d = nc.dram_tensor("wv", [DC, P, D], BF16, kind="ExternalInput")
    wo_d = nc.dram_tensor("wo", [DC, P, D], BF16, kind="ExternalInput")
    bq_d = nc.dram_tensor("bq", [P, DC], F32, kind="ExternalInput")
    bk_d = nc.dram_tensor("bk", [P, DC], F32, kind="ExternalInput")
    bo_d = nc.dram_tensor("bo", [D], F32, kind="ExternalInput")
    mb_d = nc.dram_tensor("maskb", [P, NT], F32, kind="ExternalInput")
    y_d = nc.dram_tensor("y", [NQ, D], F32, kind="ExternalOutput")

    consts = ctx.enter_context(tc.tile_pool(name="consts", bufs=1))
    big = ctx.enter_context(tc.tile_pool(name="big", bufs=1))
    ptp = ctx.enter_context(tc.tile_pool(name="ptp", bufs=5))
    rlp = ctx.enter_context(tc.tile_pool(name="rlp", bufs=6))
    yout = ctx.enter_context(tc.tile_pool(name="yout", bufs=3))

    # dummy exp first on the ACT queue: preloads the exp table-set (~2.7us)
    # while the DMAs stream
    zz = consts.tile([P, 512], BF16, tag="zz")
    nc.vector.memset(zz, 0.0)
    dume = consts.tile([P, 1], BF16, tag="dume")
    nc.scalar.activation(
        out=dume, in_=zz[:, 0:1], func=mybir.ActivationFunctionType.Exp
    )

    # weights split across both HWDGE queues (ScalarE idle until first exp)
    w_sb = {
        name: big.tile([P, DC, D], BF16, tag=f"{name}_sb", name=f"{name}_sb")
        for name in ("wq", "wk", "wv", "wo")
    }
    for name, d in (("wv", wv_d), ("wq", wq_d), ("wo", wo_d)):
        nc.scalar.dma_start(
            w_sb[name][:, :, :], d[:, :, :].rearrange("c p d -> p c d")
        )
    nc.sync.dma_start(
        w_sb["wk"][:, :, :], wk_d[:, :, :].rearrange("c p d -> p c d")
    )

    bq_sb = consts.tile([P, DC], F32, tag="bq")
    nc.sync.dma_start(bq_sb, bq_d[:, :])
    bk_sb = consts.tile([P, DC], F32, tag="bk")
    nc.sync.dma_start(bk_sb, bk_d[:, :])
    mb_sb = consts.tile([P, NT], F32, tag="mb")
    nc.sync.dma_start(mb_sb, mb_d[:, :])
    bo_sb = consts.tile([P, D], F32, tag="bo")
    bo_ap = bo_d[:]
    nc.sync.dma_start(
        bo_sb, bass.AP(tensor=bo_ap.tensor, offset=bo_ap.offset, ap=[[0, P], [1, D]])
    )
    # xnT quarters on the SP queue: V/Q/K consumers unblock incrementally
    xnT = big.tile([P, DC, S], BF16, tag="xnT")
    for tg in range(4):
        nc.sync.dma_start(
            xnT[:, :, tg * 512 : (tg + 1) * 512],
            xnT_d[:, :, tg * 512 : (tg + 1) * 512].rearrange("c p s -> p c s"),
        )

    ident = consts.tile([P, P], BF16, tag="ident")
    make_identity(nc, ident)

    qT = big.tile([P, DC, NQ], BF16, tag="qT")
    kT = big.tile([P, DC, S], BF16, tag="kT")
    vaug = big.tile([P, NT, 8 * 65], BF16, tag="vaug")
    attno = big.tile([P, NQT, D], BF16, tag="attno")
    outT = big.tile([P, DC, NQ], BF16, tag="outT")
    yA = big.tile([P, NQT, D], F32, tag="yA")

    def v_group(pool, t):
        def emit():
            vps = pool.tile([P, 512], F32, tag="pp", name=f"vps{t}")
            for dc in range(DC):
                nc.tensor.matmul(
                    vps,
                    xnT[:, dc, t * P : (t + 1) * P],
                    w_sb["wv"][:, dc, :],
                    start=(dc == 0), stop=(dc == DC - 1),
                )
            vslot = vaug[:, t, :].rearrange("p (h c) -> p h c", h=H)
            nc.vector.tensor_copy(
                out=vslot[:, :, 0:DK],
                in_=vps[:].rearrange("p (h c) -> p h c", h=H),
            )
            nc.vector.memset(vslot[:, :, DK : DK + 1], 1.0)
        return emit

    def qk_groups(pool, p):
        """Emit-closures for pair p's Q and K projections (d-chunk p)."""
        groups = []

        def proj_group(w_name, out_t, bias, gg):
            def emit():
                ps = pool.tile([P, 512], F32, tag="pp", name=f"{w_name}ps{p}_{gg}")
                for dc in range(DC):
                    nc.tensor.matmul(
                        ps,
                        w_sb[w_name][:, dc, p * P : (p + 1) * P],
                        xnT[:, dc, gg * 512 : (gg + 1) * 512],
                        start=(dc == 0), stop=(dc == DC - 1),
                    )
                nc.vector.tensor_scalar_add(
                    out=out_t[:, p, gg * 512 : (gg + 1) * 512], in0=ps,
                    scalar1=bias[:, p : p + 1],
                )
            return emit

        for qg in range(NQ // 512):
            groups.append(proj_group("wq", qT, bq_sb, qg))
        for kg in range(S // 512):
            groups.append(proj_group("wk", kT, bk_sb, kg))
        return groups

    def recips(p, pvb):
        rls = []
        for j, bank in enumerate(pvb):
            nslot = 3 if j < 2 else 2
            rl = rlp.tile([P, 3, 2], F32, tag="rl", name=f"rl{p}_{j}")
            lcols = (
                bank[:, 0 : nslot * 130]
                .rearrange("p (s h t) -> p s h t", s=nslot, h=2)[:, :, :, 64:65]
                .rearrange("p s h t -> p (s h t)")
            )
            nc.vector.reciprocal(
                out=rl[:, :nslot, :].rearrange("p s h -> p (s h)"), in_=lcols
            )
            rls.append(rl)
        return rls

    def norm_qt(p, pvb, rls, qt):
        """Normalize one query tile of pair p into attno."""
        bank = pvb[qt // 3]
        off = (qt % 3) * 130
        for hs in range(2):
            nc.vector.tensor_scalar_mul(
                out=attno[:, qt, (2 * p + hs) * DK : (2 * p + hs + 1) * DK],
                in0=bank[:, off + hs * 65 : off + hs * 65 + DK],
                scalar1=rls[qt // 3][:, qt % 3, hs : hs + 1],
            )

    def tr_qt(pool, p, qt, on_act=False):
        """Transpose one normalized query tile into outT."""
        tre = pool.tile([P, P], BF16, tag="pp", name=f"tre{p}_{qt}")
        nc.tensor.transpose(tre, attno[:, qt, p * P : (p + 1) * P], ident)
        if on_act:
            nc.scalar.copy(out=outT[:, p, qt * P : (qt + 1) * P], in_=tre)
        else:
            nc.vector.tensor_copy(out=outT[:, p, qt * P : (qt + 1) * P], in_=tre)

    def oprojA_group(pool, qt):
        """Output projection over d-chunks 0..2 (ready before pair 3 ends);
        bias folded in so the tail only adds the last chunk's contribution."""
        def emit():
            ps = pool.tile([P, D], F32, tag="pp", name=f"oA{qt}")
            for dc in range(DC - 1):
                nc.tensor.matmul(
                    ps,
                    outT[:, dc, qt * P : (qt + 1) * P],
                    w_sb["wo"][:, dc, :],
                    start=(dc == 0), stop=(dc == DC - 2),
                )
            nc.vector.tensor_tensor(
                out=yA[:, qt, :], in0=ps, in1=bo_sb, op=mybir.AluOpType.add
            )
        return emit

    # prologue: warmup + V for the first chunks + pair-0 Q/K
    with tc.tile_pool(name="prol", bufs=3, space="PSUM") as prol:
        wps = prol.tile([P, 512], F32, tag="warm", bufs=1)
        for _ in range(N_WARM):
            nc.tensor.matmul(wps, zz[:, 0:P], zz, start=True, stop=True)
        for t in range(2):
            v_group(prol, t)()
        for g in qk_groups(prol, 0):
            g()

    # ---------------- attention, everything else in its shadow ----------------
    with tc.tile_pool(name="att", bufs=1, space="PSUM") as att:
        sc = [
            att.tile([P, NQ], F32, tag=f"sc{hs}", name=f"sc{hs}") for hs in (0, 1)
        ]
        pending = []
        carry = None
        for p in range(PAIRS):
            pvb = None  # allocated at c==2, AFTER the previous pair's
            # evacuation burst is emitted — the 3-buffer ring's WAR edges
            # only see already-emitted readers of the old tiles.
            if p == 0:
                pending += [v_group(att, t) for t in range(2, NT)]
            if p + 1 < PAIRS:
                pending += qk_groups(att, p + 1)
            pts = [None] * NT
            for c in range(NT):
                # 4 score matmuls; hs-adjacent issue order so the two
                # 64-row tiles (rows 0-63 / 64-127) overlap in the array.
                for qg in range(NQ // 512):
                    for hs in range(2):
                        nc.tensor.matmul(
                            sc[hs][:, qg * 512 : (qg + 1) * 512],
                            kT[hs * DK : (hs + 1) * DK, p, c * P : (c + 1) * P],
                            qT[hs * DK : (hs + 1) * DK, p, qg * 512 : (qg + 1) * 512],
                            start=True, stop=True,
                        )
                pt = ptp.tile([P, 2 * NQ], BF16, tag="pt")
                pts[c] = pt
                for hs in range(2):
                    nc.scalar.activation(
                        out=pt[:, hs * NQ : (hs + 1) * NQ], in_=sc[hs],
                        func=mybir.ActivationFunctionType.Exp,
                        bias=mb_sb[:, c : c + 1], scale=1.0 / math.sqrt(DK),
                    )
                if carry is not None:
                    cpt, cb, cp = carry
                    if c == 0:
                        # previous pair's PV tail chunk
                        _pv_chunk(nc, cpt, vaug, cb, cp, NT - 1)
                    elif c == 1:
                        carry_rls = recips(cp, cb)
                        for qt in range(4):
                            norm_qt(cp, cb, carry_rls, qt)
                    elif c == 2:
                        for qt in range(4, NQT):
                            norm_qt(cp, cb, carry_rls, qt)
                    elif c in (3, 4):
                        for qt in range((c - 3) * 4, (c - 2) * 4):
                            tr_qt(att, cp, qt)
                        if c == 4:
                            carry = None
                            if p == PAIRS - 1:
                                # outT d-chunks 0..2 now fully written
                                pending = [
                                    oprojA_group(att, qt) for qt in range(NQT)
                                ] + pending
                # PV lags exp by 3 chunks: the PE FIFO never stalls on the
                # previous pair's bank evacuation.
                if c >= 3:
                    if pvb is None:
                        pvb = [
                            att.tile([P, 512], F32, tag="pvb", bufs=3,
                                     name=f"pvb{p}_{j}")
                            for j in range(3)
                        ]
                    _pv_chunk(nc, pts[c - 3], vaug, pvb, p, c - 3)
                    pts[c - 3] = None
                # drain deferred work, faster when backlogged
                npop = 2 if (p == PAIRS - 1 or len(pending) > NT - 1 - c) else 1
                for _ in range(min(npop, len(pending))):
                    pending.pop(0)()
            for ct in (NT - 3, NT - 2):
                _pv_chunk(nc, pts[ct], vaug, pvb, p, ct)
            if p + 1 < PAIRS:
                carry = (pts[NT - 1], pvb, p)
            else:
                # last pair: tail chunk, then per-qt pipeline to the output
                _pv_chunk(nc, pts[NT - 1], vaug, pvb, p, NT - 1)
                rls = recips(p, pvb)
                for qt in range(NQT):
                    norm_qt(p, pvb, rls, qt)
                    tr_qt(att, p, qt, on_act=True)
                    po = att.tile([P, D], F32, tag="pp", name=f"po{qt}")
                    nc.tensor.matmul(
                        po,
                        outT[:, DC - 1, qt * P : (qt + 1) * P],
                        w_sb["wo"][:, DC - 1, :],
                        start=True, stop=True,
                    )
                    yt = yout.tile([P, D], F32, tag="yt")
                    nc.vector.tensor_tensor(
                        out=yt, in0=po, in1=yA[:, qt, :], op=mybir.AluOpType.add
                    )
                    nc.sync.dma_start(y_d[qt * P : (qt + 1) * P, :], yt)


def _pv_chunk(nc, pt, vaug, pvb, p, c):
    """P@[V|1] matmuls for chunk c of head-pair p: 8 query tiles x 2 heads,
    accumulated over chunks into the packed PSUM banks."""
    for qt in range(NQT):
        bank = pvb[qt // 3]
        off = (qt % 3) * 130
        for hs in range(2):
            h = 2 * p + hs
            # start=True clears has_written for the WHOLE bank, so only the
            # first packed region per bank may use it; the others rely on
            # overwrite-when-bit-clear for their first chunk.
            nc.tensor.matmul(
                bank[:, off + hs * 65 : off + (hs + 1) * 65],
                pt[:, hs * NQ + qt * P : hs * NQ + (qt + 1) * P],
                vaug[:, c, h * 65 : (h + 1) * 65],
                start=(c == 0 and qt % 3 == 0 and hs == 0),
                stop=(c == NT - 1),
                skip_group_check=True,
            )


_NC = None


def _get_nc():
    global _NC
    if _NC is None:
        from contextlib import ExitStack

        nc = bacc.Bacc(None, target_bir_lowering=False)
        with tile.TileContext(nc) as tc, ExitStack() as ctx:
            _emit(tc, ctx)
        nc.compile()
        _NC = nc
    return _NC


def kernel(
    inputs, input_lengths, pos_embed, ln_gamma, ln_beta,
    Wq, bq, Wk, bk, Wv, bv, Wo, bo,
):
    x = np.ascontiguousarray(np.asarray(inputs, np.float32))
    lengths = np.asarray(input_lengths, np.int32)
    g = np.asarray(ln_gamma, np.float32)
    be = np.asarray(ln_beta, np.float32)
    Wq = np.asarray(Wq, np.float32); bq = np.asarray(bq, np.float32)
    Wk = np.asarray(Wk, np.float32); bk = np.asarray(bk, np.float32)
    Wv = np.asarray(Wv, np.float32); bv = np.asarray(bv, np.float32)
    Wo = np.asarray(Wo, np.float32); bo = np.asarray(bo, np.float32)

    # LayerNorm on host (eps=1e-5), fp32, then bf16 d-major per core.
    mu = x.mean(-1, keepdims=True)
    xc = x - mu
    var = np.mean(xc * xc, axis=-1, keepdims=True)
    xn = (xc / np.sqrt(var + 1e-5)) * g + be

    def chunks(w):  # [D, D] -> [DC, P, D] contiguous row chunks of W.T
        return np.ascontiguousarray(w.T.astype(np_bf16).reshape(DC, P, D))

    wq_h, wk_h, wv_h, wo_h = chunks(Wq), chunks(Wk), chunks(Wv), chunks(Wo)
    bq_h = np.ascontiguousarray(bq.reshape(DC, P).T)
    bk_h = np.ascontiguousarray(bk.reshape(DC, P).T)
    # V bias passes through softmax (rows sum to 1) -> fold into output bias.
    bo_h = np.ascontiguousarray(bo + bv @ Wo.T)

    maskb = np.where(np.arange(S)[None, :] < lengths[:, None], 0.0, NEG).astype(
        np.float32
    )

    nc = _get_nc()
    in_maps = []
    core_assign = []
    for b in range(B):
        for gq in range(2):
            order = np.r_[gq * NQ : (gq + 1) * NQ, (1 - gq) * NQ : (2 - gq) * NQ]
            in_maps.append(
                {
                    "xnT": np.ascontiguousarray(
                        xn[b][order].T.astype(np_bf16).reshape(DC, P, S)
                    ),
                    "wq": wq_h, "wk": wk_h, "wv": wv_h, "wo": wo_h,
                    "bq": bq_h, "bk": bk_h, "bo": bo_h,
                    "maskb": np.ascontiguousarray(maskb[b][order].reshape(NT, P).T),
                }
            )
            core_assign.append((b, gq))

    global _LAST_IN_MAPS
    _LAST_IN_MAPS = in_maps
    res = run_bass_kernel_spmd(nc, in_maps, core_ids=list(range(8)))

    y = np.empty((B, S, D), np.float32)
    for i, (b, gq) in enumerate(core_assign):
        y[b, gq * NQ : (gq + 1) * NQ] = res.results[i]["y"]
    return y
